# revision 30
# baseline (speedup 1.0000x reference)
"""Trainium2 Bass kernel: per-channel broadcast multiply (ChannelMultiplier).

out[n, c, h, w] = x[n, c, h, w] * multiplier[c]

x: (32, 256, 56, 56) f32, multiplier: (256,) f32.

Sharding: data-parallel over the batch dim N across 8 NeuronCores
(4 batches per core); the 1 KB multiplier is replicated to every core.

Per-core layout: the local shard (4, 256, 56, 56) is viewed row-major as
(1024, 3136); row r = n*256 + c is one (n, c) image plane of 3136
contiguous floats.  Grouping rows as (n, h, p) with h = channel half
(C = 256 = 2*128) puts a FIXED channel on each SBUF partition, so a whole
[128, 3136] tile is scaled by a single per-partition vector (a half of
`multiplier`) in ONE vector-engine tensor_scalar_mul (runs in the 2x
dual-read-port fp32 mode, ~1.85 us per 1.6 MiB tile).

The kernel is HBM-bandwidth-bound: 12.85 MiB in + 12.85 MiB out per core.
Measured on TRN2 via axon it streams at ~430 GB/s (SBUF-AXI fabric rate;
each axon core has its HBM domain to itself), so the floor is ~60 us of
data movement plus ~7 us fixed kernel preamble (sem-clear barriers, engine
table loads) and ~2.6 us tail drain -> ~74 us measured.

Schedule notes (keeps every engine/DMA instruction at <= 1 semaphore wait,
minimizing EventSemaphore splits and SP dispatch stalls):
  * 7 full tiles + 2 half tiles of the last (n, h) to trim the tail;
    every tile has its own SBUF slot (no WAR waits, ~100 KB/partition);
  * all loads are traced and force-ordered before all stores, so loads
    grab the first HWDGE completion lanes with no in-stream lane-FIFO
    stalls on the dispatching sequencer;
  * loads and stores alternate between the two HWDGE rings (SP and ACT)
    for parallel dispatch;
  * the tiny scale DMA goes through SWDGE (gpsimd), a separate lane pool;
  * each DVE multiply waits only on its own load's DMA lane and fully
    shadows the load's write (same access pattern), so each store waits
    only on the DVE semaphore;
  * the per-partition scalar operand of TensorScalar is read in the
    engine's setup phase (a pointer-read hazard needing one sem wait at
    the first consumer), so a warm-up op takes that wait once.
"""

import numpy as np

import concourse.bacc as bacc
import concourse.bass as bass
import concourse.mybir as mybir
import concourse.tile as tile_mod
from concourse.bass_utils import run_bass_kernel_spmd
from concourse.tile import TileContext

N, C, H, W = 32, 256, 56, 56
N_CORES = 8
NL = N // N_CORES  # batches per core
P = 128  # SBUF partitions
F = H * W  # 3136 contiguous floats per (n, c) row
ROWS = NL * C  # 1024 rows per core
HALVES = C // P  # 2 channel halves
FSPLIT = 2  # f-dim slices for the final (batch, half) tile (tail trim)
STORE_DELAY = 2  # gate store t on mul t+STORE_DELAY (load-priority scheduling)
# Tile plan: 7 full [128, F] tiles + FSPLIT slices of the last one.
# (n, h, s, nsplit): f-slice s of nsplit for batch n, channel half h.
TILE_PLAN = [
    (n, h, 0, 1) for n in range(NL) for h in range(HALVES)
][:-1] + [(NL - 1, HALVES - 1, s, FSPLIT) for s in range(FSPLIT)]

_NC_CACHE: list = [None]


def _build() -> bass.Bass:
    # Bacc (not raw Bass): its finalize() runs generate_event_semaphores,
    # which splits multi-wait sync_info into InstEventSemaphore chains —
    # engine ISA words only carry one semaphore wait each.
    nc = bacc.Bacc()
    x = nc.declare_dram_parameter("x", [ROWS, F], mybir.dt.float32, isOutput=False)
    mult = nc.declare_dram_parameter("multiplier", [C], mybir.dt.float32, isOutput=False)
    y = nc.declare_dram_parameter("y", [ROWS, F], mybir.dt.float32, isOutput=True)

    # [n, h, p, f]: channels h*128..h*128+127 of batch n, one channel per
    # partition; f-slices are taken with a plain column slice.
    xv = x.rearrange("(n h p) f -> n h p f", h=HALVES, p=P)
    yv = y.rearrange("(n h p) f -> n h p f", h=HALVES, p=P)
    # [p, h]: column h holds multiplier[h*128 + p].
    mv = mult.rearrange("(h p) -> p h", h=HALVES)

    with TileContext(nc) as tc:
        with (
            tc.tile_pool(name="scale", bufs=1) as spool,
            tc.tile_pool(name="data", bufs=1) as pool,
        ):
            # Scale staging: SWDGE DMA -> sc, DVE copy -> sc2 (takes the
            # DMA wait), warm-up TensorScalar consumes sc2's pointer
            # (takes the same-engine pointer-read hazard wait).
            sc = spool.tile([P, HALVES], mybir.dt.float32, tag="sc")
            nc.gpsimd.dma_start(out=sc[:, :], in_=mv)
            sc2 = spool.tile([P, HALVES], mybir.dt.float32, tag="sc2")
            nc.vector.tensor_copy(out=sc2[:, :], in_=sc[:, :])
            scr = spool.tile([P, HALVES], mybir.dt.float32, tag="scr")
            warm = nc.vector.tensor_scalar_mul(scr[:, :], sc2[:, :], sc2[:, 0:1])

            # All loads first: they dispatch back-to-back from SP with no
            # waits, so DMA bandwidth is busy from t=0.  Ordering deps force
            # every store after the last load in the scheduler's order, so
            # loads take the first HWDGE lanes (no in-stream lane stalls on
            # SP) and each store's lane-FIFO wait is on a load that already
            # completed.
            tiles = []
            loads = []
            for t, (n, h, s, nsplit) in enumerate(TILE_PLAN):
                fs = F // nsplit
                nslots = sum(1 for p_ in TILE_PLAN if p_[3] == nsplit)
                tile = pool.tile(
                    [P, fs], mybir.dt.float32, tag=f"data{nsplit}", bufs=nslots
                )
                # Alternate the two HWDGE rings (SP / ACT) so descriptor
                # generation for concurrent transfers runs on both.
                eng = nc.sync if t % 2 == 0 else nc.scalar
                ld = eng.dma_start(
                    out=tile[:, :], in_=xv[n, h][:, s * fs : (s + 1) * fs]
                )
                loads.append(ld)
                tiles.append(tile)
            last_load = loads[-1]

            muls = []
            for (n, h, s, nsplit), tile in zip(TILE_PLAN, tiles):
                mul = nc.vector.tensor_scalar_mul(
                    tile[:, :], tile[:, :], sc2[:, h : h + 1]
                )
                # Keep the warm-up ahead of every scalar-pointer consumer
                # in the DVE stream (ordering only, no semaphore).
                tile_mod.add_dep_helper(
                    mul.ins, warm.ins, sync=False, reason="scale ptr hazard warm-up"
                )
                muls.append(mul)

            for t, ((n, h, s, nsplit), tile) in enumerate(zip(TILE_PLAN, tiles)):
                fs = F // nsplit
                # Store on the opposite ring from this tile's load.
                eng = nc.scalar if t % 2 == 0 else nc.sync
                st = eng.dma_start(
                    out=yv[n, h][:, s * fs : (s + 1) * fs], in_=tile[:, :]
                )
                tile_mod.add_dep_helper(
                    st.ins, last_load.ins, sync=False, reason="stores after loads"
                )
                # Gate each store on the mul STORE_DELAY tiles ahead: early
                # on both HWDGE rings then carry only loads (loads get the
                # full HBM bandwidth, finishing sooner), and the final
                # mul+store chain hides behind the queued store backlog.
                gate = muls[min(t + STORE_DELAY, len(muls) - 1)]
                if gate is not muls[t]:
                    tile_mod.add_dep_helper(
                        st.ins, gate.ins, sync=True, reason="delay store dispatch"
                    )
    nc.finalize()
    return nc


def _get_nc() -> bass.Bass:
    if _NC_CACHE[0] is None:
        _NC_CACHE[0] = _build()
    return _NC_CACHE[0]


def kernel(x: np.ndarray, multiplier: np.ndarray) -> np.ndarray:
    x = np.ascontiguousarray(x, dtype=np.float32)
    multiplier = np.ascontiguousarray(multiplier, dtype=np.float32)
    assert x.shape == (N, C, H, W), x.shape
    assert multiplier.shape == (C,), multiplier.shape

    xr = x.reshape(N_CORES, ROWS, F)
    in_maps = [{"x": xr[i], "multiplier": multiplier} for i in range(N_CORES)]
    res = run_bass_kernel_spmd(_get_nc(), in_maps, list(range(N_CORES)))
    out = np.concatenate(
        [r["y"].reshape(NL, C, H, W) for r in res.results], axis=0
    )
    return out
